# revision 1
# baseline (speedup 1.0000x reference)
"""Self-contained Trainium2 Bass kernel for nn_BipartiteDataEncoder.\nAccepts full inputs, shards across 8 NeuronCores internally."""
"""Host-side preprocessing: sharding, permutations, edge schedules.

Terminology:
  - var nodes are dst of direction 'cv' (cons->var); cons nodes are dst of 'vc'.
  - Nodes are relabeled: node -> core (node % 8) -> degree-sorted rank within core.
    Padded shard sizes: SV=25088 (var, 196 blocks), SC=12544 (cons, 98 blocks).
  - Source tables are windowed (WIN=25088 rows) so int16 local indices fit.
  - Edges of a core/direction are grouped by (dst block b, src window w), padded
    to whole 128-edge tiles; tile counts T[b,w] shared across cores (SPMD).
  - Tile memory order: for each group g of G blocks: for w: for b in g: tiles.
"""
import numpy as np

NCONS, NVAR, NEDGE, EMB = 100_000, 200_000, 2_000_000, 64
NCORE = 8
BLK = 128
SV = 25_088   # var shard (padded), 196 blocks
SC = 12_544   # cons shard (padded), 98 blocks
WIN = 25_088
VP, CP = SV * NCORE, SC * NCORE      # padded table sizes
WV, WC = VP // WIN, CP // WIN        # 8 var windows, 4 cons windows
G = 8                                # blocks per psum-group (even)


def node_permutation(n_nodes, shard_size, deg):
    """Return pid[node] mapping node -> padded id (core*shard_size + rank),
    core = node % NCORE, rank = degree-sorted (desc) position within core."""
    pid = np.empty(n_nodes, dtype=np.int64)
    for k in range(NCORE):
        nodes = np.arange(k, n_nodes, NCORE)
        order = np.argsort(-deg[nodes], kind="stable")
        pid[nodes[order]] = k * shard_size + np.arange(len(nodes))
    return pid


def build_schedule(src_pid_of_edge, dst_pid_of_edge, shard_size, n_windows):
    """Build the tiled edge schedule for one direction.

    Returns dict with:
      T        [NBLK, W] shared tile counts
      tile_of  [NBLK, W] start tile index of each (b, w) run (tile order)
      ntiles   total tiles
      groups   list of (blocks list, [(w, tile_start, ntiles_run), ...])
      chunks   list of gather chunks (w, tile_start, ntiles) with ntiles*128 <= GSZ
      idx16    [NCORE, ntiles*128] int16 source-local row ids (pad 0)
      dstloc   [NCORE, ntiles*128] float32 dst-in-block ids (pad -1)
    """
    nblk = shard_size // BLK
    dst_core = dst_pid_of_edge // shard_size
    dst_loc = dst_pid_of_edge % shard_size
    b_of = dst_loc // BLK
    din = dst_loc % BLK
    w_of = src_pid_of_edge // WIN
    src_loc = (src_pid_of_edge % WIN).astype(np.int64)

    # per-core counts -> shared T
    counts = np.zeros((NCORE, nblk, n_windows), dtype=np.int64)
    key_all = (dst_core * nblk + b_of) * n_windows + w_of
    cnt_flat = np.bincount(key_all, minlength=NCORE * nblk * n_windows)
    counts = cnt_flat.reshape(NCORE, nblk, n_windows)
    T = np.ceil(counts.max(axis=0) / BLK).astype(np.int64)      # [nblk, W]

    # tile order: for g: for w: for b in g — adaptive group sizes
    TILE_BUDGET = 104
    MAXB = 30
    tile_of = np.zeros((nblk, n_windows), dtype=np.int64)
    per_block = T.sum(axis=1)
    groups = []
    t = 0
    b = 0
    while b < nblk:
        blocks = [b]
        tot = per_block[b]
        b += 1
        while b < nblk and len(blocks) < MAXB and tot + per_block[b] <= TILE_BUDGET:
            tot += per_block[b]
            blocks.append(b)
            b += 1
        runs = []
        for w in range(n_windows):
            run_start = t
            for bb in blocks:
                tile_of[bb, w] = t
                t += T[bb, w]
            if t > run_start:
                runs.append((w, run_start, t - run_start))
        groups.append((blocks, runs))
    ntiles = t

    # per-core edge placement
    idx16 = np.zeros((NCORE, ntiles * BLK), dtype=np.int16)
    dstloc = np.full((NCORE, ntiles * BLK), -1.0, dtype=np.float32)
    order = np.lexsort((w_of, b_of, dst_core))
    sc, sb, sw = dst_core[order], b_of[order], w_of[order]
    ssrc, sdin = src_loc[order], din[order]
    key = (sc * nblk + sb) * n_windows + sw
    first = np.r_[True, key[1:] != key[:-1]]
    grp_start = np.maximum.accumulate(np.where(first, np.arange(len(key)), 0))
    rank = np.arange(len(key)) - grp_start
    pos = tile_of[sb, sw] * BLK + rank
    idx16[sc, pos] = ssrc.astype(np.int16)
    dstloc[sc, pos] = sdin.astype(np.float32)

    # gather chunks: split each (w) run at GSZ boundaries
    GSZ_TILES = 8192 // BLK
    chunks = []
    for blocks, runs in groups:
        for (w, ts, n) in runs:
            s = ts
            while s < ts + n:
                m = min(GSZ_TILES, ts + n - s)
                chunks.append((w, s, m))
                s += m
    return dict(T=T, tile_of=tile_of, ntiles=ntiles, groups=groups,
                chunks=chunks, idx16=idx16, dstloc=dstloc, counts=counts)


def wrap16(flat_i16):
    """dma_gather index wrap: idx i -> [i%16, i//16], replicated 8x to 128 partitions."""
    w = flat_i16.reshape(-1, 16).T
    return np.ascontiguousarray(np.tile(w, (8, 1)))


def preprocess(inputs):
    """Full host-side preprocessing. Returns dict of everything the kernel needs."""
    inp = {k: np.asarray(v) for k, v in inputs.items()}
    row = inp["edge_index"][0].astype(np.int64)
    col = inp["edge_index"][1].astype(np.int64)

    deg_v = np.bincount(col, minlength=NVAR)     # cv in-degree of var
    deg_c = np.bincount(row, minlength=NCONS)    # vc in-degree of cons
    pid_v = node_permutation(NVAR, SV, deg_v)
    pid_c = node_permutation(NCONS, SC, deg_c)
    row_p = pid_c[row]
    col_p = pid_v[col]

    sched_cv = build_schedule(row_p, col_p, SV, WC)   # dst var, src cons windows
    sched_vc = build_schedule(col_p, row_p, SC, WV)   # dst cons, src var windows

    # permuted, padded, transposed node inputs with ones row (host)
    def padT(x, pid, P):
        xt = np.zeros((P, x.shape[1]), dtype=np.float32)
        xt[pid] = x
        return np.ascontiguousarray(xt.T)

    cons_xT = padT(inp["cons_x"].astype(np.float32), pid_c, CP)      # [5, CP]
    var_xT = padT(inp["var_x"].astype(np.float32), pid_v, VP)        # [19, VP]
    breakT = padT(inp["break_indicator"].astype(np.float32), pid_v, VP)  # [1, VP]

    # fold prenorm into W1/b1 (x+shift)*scale @ W1 + b1
    def fold(W1, b1, shift, scale):
        W1f = scale[:, None] * W1
        b1f = b1 + (shift * scale) @ W1
        return W1f.astype(np.float32), b1f.astype(np.float32)

    cW1f, cb1f = fold(inp["cons_W1"], inp["cons_b1"], inp["cons_pn_shift"], inp["cons_pn_scale"])
    vW1f, vb1f = fold(inp["var_W1"], inp["var_b1"], inp["var_pn_shift"], inp["var_pn_scale"])

    # per-core recips and break sums (padded-local indexing)
    deg_v_p = np.zeros(VP, dtype=np.float32); deg_v_p[pid_v] = deg_v
    deg_c_p = np.zeros(CP, dtype=np.float32); deg_c_p[pid_c] = deg_c
    recip_v = (1.0 / np.maximum(deg_v_p, 1.0)).reshape(NCORE, SV)
    recip_c = (1.0 / np.maximum(deg_c_p, 1.0)).reshape(NCORE, SC)
    brk = inp["break_indicator"].astype(np.float64)[:, 0]
    bsum = np.zeros(CP, dtype=np.float64)
    np.add.at(bsum, row_p, brk[col])
    bsum_c = bsum.astype(np.float32).reshape(NCORE, SC)

    return dict(
        pid_v=pid_v, pid_c=pid_c, row_p=row_p, col_p=col_p,
        sched_cv=sched_cv, sched_vc=sched_vc,
        cons_xT=cons_xT, var_xT=var_xT, breakT=breakT,
        cW1f=cW1f, cb1f=cb1f, cW2=inp["cons_W2"].astype(np.float32), cb2=inp["cons_b2"].astype(np.float32),
        vW1f=vW1f, vb1f=vb1f, vW2=inp["var_W2"].astype(np.float32), vb2=inp["var_b2"].astype(np.float32),
        breakW=inp["break_W"].astype(np.float32),                     # [1, 64]
        Wl_cv=inp["Wl_cv"].astype(np.float32), bl_cv=inp["bl_cv"].astype(np.float32),
        Wr_cv=inp["Wr_cv"].astype(np.float32),
        Wl_vc=inp["Wl_vc"].astype(np.float32), bl_vc=inp["bl_vc"].astype(np.float32),
        Wr_vc=inp["Wr_vc"].astype(np.float32),
        recip_v=recip_v, recip_c=recip_c, bsum_c=bsum_c,
    )


# ---- kernel builder ----
import contextlib
import numpy as np
import ml_dtypes
import concourse.bacc as bacc
import concourse.bass as bass
import concourse.mybir as mybir
import concourse.tile as tile
from concourse.masks import make_identity

F32 = mybir.dt.float32
BF16 = mybir.dt.bfloat16
I16 = mybir.dt.int16
RELU = mybir.ActivationFunctionType.Relu
COPY = mybir.ActivationFunctionType.Copy
ADD = mybir.AluOpType.add
MULT = mybir.AluOpType.mult
ISEQ = mybir.AluOpType.is_equal
STRIPE = 8192
OUT_W = SV // 2
GCHUNK = 26            # tiles per gather instruction


def bf(x):
    return np.asarray(np.asarray(x, dtype=np.float32), dtype=ml_dtypes.bfloat16)


def bcast(row_ap, nparts):
    """[1, N] AP -> [nparts, N] partition-broadcast."""
    return row_ap.partition_broadcast(nparts).squeeze(1)


def build(P, phases="EVC"):
    nc = bacc.Bacc("TRN2", target_bir_lowering=False,
                   dynamic_dma_scratch_size=65536, num_swdge_queues=4)
    scv, svc = P["sched_cv"], P["sched_vc"]
    NT_CV, NT_VC = scv["ntiles"], svc["ntiles"]

    def inp(name, shape, dt):
        return nc.dram_tensor(name, shape, dt, kind="ExternalInput")

    cxT = inp("cxT", [6, CP], BF16)
    vxT = inp("vxT", [20, VP], BF16)
    o_vxT = inp("o_vxT", [20, SV], BF16)
    o_cxT = inp("o_cxT", [6, SC], BF16)
    o_brk = inp("o_brk", [1, SV], BF16)
    c_l1 = inp("c_l1", [6, 64], BF16)
    c_l2 = inp("c_l2", [65, 64], BF16)
    v_l1 = inp("v_l1", [20, 64], BF16)
    v_l2 = inp("v_l2", [65, 64], BF16)
    w_in = {}
    for nm in ("wl_vc", "wr_vc", "wl_cv0", "wr_cv0", "wl_cv1", "wr_cv1"):
        w_in[nm] = inp(nm, [64, 64], BF16)
    bl_in = {}
    for nm in ("bl_vc", "bl_cv0", "bl_cv1"):
        bl_in[nm] = inp(nm, [64, 1], F32)
    brkw_b = inp("brkw_b", [64, 1], BF16)
    brkw_f = inp("brkw_f", [64, 1], F32)
    iota = inp("iota", [1, 128], BF16)
    recv = inp("recv", [1, SV], F32)
    recc = inp("recc", [1, SC], F32)
    bsum = inp("bsum", [1, SC], F32)
    vc_idx = inp("vc_idx", [128, NT_VC * 8], I16)
    vc_dst = inp("vc_dst", [128, NT_VC], F32)
    cv_idx = inp("cv_idx", [128, NT_CV * 8], I16)
    cv_dst = inp("cv_dst", [128, NT_CV], F32)

    out = nc.dram_tensor("out", [128, OUT_W], F32, kind="ExternalOutput")
    dbg = nc.dram_tensor("dbg", [SC, 64], BF16, kind="ExternalOutput")

    var_tab = nc.dram_tensor("var_tab", [VP, 128], BF16)
    cv_tab = nc.dram_tensor("cv_tab", [CP, 128], BF16)
    vroot = nc.dram_tensor("vroot", [64, SV], BF16)
    croot = nc.dram_tensor("croot", [64, SC], BF16)
    ag_in = nc.dram_tensor("ag_in", [SC, 64], BF16)
    ag_out = nc.dram_tensor("ag_out", [CP, 64], BF16, addr_space="Shared")

    vwin_w = [[] for _ in range(WV)]
    cwin_w = [[] for _ in range(WC)]
    ag_writes = []
    root_w = {'vroot': [], 'croot': []}

    with tile.TileContext(nc) as tc, contextlib.ExitStack() as stk:
        cpool = stk.enter_context(tc.tile_pool(name="consts", bufs=1))
        t_iota = cpool.tile([128, 128], BF16)
        nc.sync.dma_start(out=t_iota[:], in_=iota[0:1, :].partition_broadcast(128).squeeze(1))
        t_w = {}
        for nm, h in w_in.items():
            t_w[nm] = cpool.tile([64, 64], BF16, tag=nm, name="t_" + nm)
            nc.sync.dma_start(out=t_w[nm][:], in_=h[:])
        t_bl = {}
        for nm, h in bl_in.items():
            t_bl[nm] = cpool.tile([64, 1], F32, tag=nm, name="tb_" + nm)
            nc.sync.dma_start(out=t_bl[nm][:], in_=h[:])
        t_brkb = cpool.tile([64, 1], BF16, tag="brkb")
        nc.sync.dma_start(out=t_brkb[:], in_=brkw_b[:])
        t_brkf = cpool.tile([64, 1], F32, tag="brkf")
        nc.sync.dma_start(out=t_brkf[:], in_=brkw_f[:])
        t_ident = cpool.tile([64, 64], BF16)
        make_identity(nc, t_ident[:])
        t_l1c = cpool.tile([6, 64], BF16, tag="l1c")
        nc.sync.dma_start(out=t_l1c[:], in_=c_l1[:])
        t_l1v = cpool.tile([20, 64], BF16, tag="l1v")
        nc.sync.dma_start(out=t_l1v[:], in_=v_l1[:])
        t_l2c = cpool.tile([65, 64], BF16, tag="l2c")
        nc.sync.dma_start(out=t_l2c[:], in_=c_l2[:])
        t_l2v = cpool.tile([65, 64], BF16, tag="l2v")
        nc.sync.dma_start(out=t_l2v[:], in_=v_l2[:])

        # ---------- Phase E ----------
        def embed_tables(xT_h, l1_t, l2_t, ncols, table, dup, win_writes):
            kin = xT_h.shape[0]
            with tc.tile_pool(name="emb", bufs=2) as ep, \
                 tc.tile_pool(name="emb_ps", bufs=2, space="PSUM") as pp, \
                 tc.tile_pool(name="emb_out", bufs=3) as op:
                for s0 in range(0, ncols, STRIPE):
                    sw = min(STRIPE, ncols - s0)
                    xs = ep.tile([kin, STRIPE], BF16, tag="xs")
                    nc.sync.dma_start(out=xs[:, :sw], in_=xT_h[:, s0:s0 + sw])
                    h1 = ep.tile([65, STRIPE], BF16, tag="h1")
                    nc.vector.memset(h1[64:65, :], 1.0)
                    for c0 in range(0, sw, 512):
                        ps = pp.tile([64, 512], F32, tag="ps1")
                        nc.tensor.matmul(ps[:], lhsT=l1_t[:], rhs=xs[:, c0:c0 + 512],
                                         start=True, stop=True)
                        nc.scalar.activation(h1[0:64, c0:c0 + 512], ps[:], RELU)
                    for c0 in range(0, sw, 512):
                        ps2 = pp.tile([128, 256], F32, tag="ps2")
                        for j in range(4):
                            cc = c0 + j * 128
                            nc.tensor.matmul(ps2[:, j * 64:(j + 1) * 64],
                                             lhsT=h1[:, cc:cc + 128], rhs=l2_t[:],
                                             start=True, stop=True)
                        ot = op.tile([128, 256], BF16, tag="ot")
                        nc.scalar.activation(ot[:], ps2[:], RELU)
                        r0 = s0 + c0
                        insts = [nc.sync.dma_start(
                            out=table[r0:r0 + 512, 0:64].rearrange("(a p) f -> p a f", p=128),
                            in_=ot[:].rearrange("p (a f) -> p a f", a=4))]
                        if dup:
                            insts.append(nc.sync.dma_start(
                                out=table[r0:r0 + 512, 64:128].rearrange("(a p) f -> p a f", p=128),
                                in_=ot[:].rearrange("p (a f) -> p a f", a=4)))
                        for w in range(r0 // WIN, min((r0 + 511) // WIN + 1, len(win_writes))):
                            win_writes[w].extend(i.ins for i in insts)

        def embed_own(xT_h, l1_t, l2_t, ncols, root, brk, wlist):
            kin = xT_h.shape[0]
            with tc.tile_pool(name="own", bufs=2) as ep, \
                 tc.tile_pool(name="own_ps", bufs=2, space="PSUM") as pp, \
                 tc.tile_pool(name="own_out", bufs=3) as op:
                for s0 in range(0, ncols, STRIPE):
                    sw = min(STRIPE, ncols - s0)
                    xs = ep.tile([kin, STRIPE], BF16, tag="xs")
                    nc.sync.dma_start(out=xs[:, :sw], in_=xT_h[:, s0:s0 + sw])
                    h1 = ep.tile([65, STRIPE], BF16, tag="h1")
                    nc.vector.memset(h1[64:65, :], 1.0)
                    brt = None
                    if brk is not None:
                        brt = ep.tile([64, STRIPE], BF16, tag="brt")
                        nc.sync.dma_start(out=brt[:, :sw],
                                          in_=brk[0:1, s0:s0 + sw].partition_broadcast(64).squeeze(1))
                    for c0 in range(0, sw, 512):
                        ps = pp.tile([64, 512], F32, tag="ps1")
                        nc.tensor.matmul(ps[:], lhsT=l1_t[:], rhs=xs[:, c0:c0 + 512],
                                         start=True, stop=True)
                        nc.scalar.activation(h1[0:64, c0:c0 + 512], ps[:], RELU)
                    for c0 in range(0, sw, 512):
                        cw = min(512, ncols - (s0 + c0))
                        ps2 = pp.tile([64, 512], F32, tag="ps2")
                        nc.tensor.matmul(ps2[:], lhsT=l2_t[:], rhs=h1[:, c0:c0 + 512],
                                         start=True, stop=True)
                        ot = op.tile([64, 512], BF16, tag="ot")
                        nc.scalar.activation(ot[:], ps2[:], RELU)
                        if brk is not None:
                            tmp = op.tile([64, 512], BF16, tag="tmp")
                            nc.vector.tensor_scalar(
                                out=tmp[:], in0=brt[:, c0:c0 + 512],
                                scalar1=t_brkf[:], scalar2=None, op0=MULT)
                            nc.vector.tensor_tensor(out=ot[:], in0=ot[:], in1=tmp[:], op=ADD)
                        wlist.append(nc.sync.dma_start(out=root[:, s0 + c0:s0 + c0 + cw], in_=ot[:, :cw]).ins)

        with nc.named_scope("embed"):
            embed_tables(vxT, t_l1v, t_l2v, VP, var_tab, True, vwin_w)
            embed_tables(cxT, t_l1c, t_l2c, CP, cv_tab, False, cwin_w)
            embed_own(o_vxT, t_l1v, t_l2v, SV, vroot, o_brk, root_w['vroot'])
            embed_own(o_cxT, t_l1c, t_l2c, SC, croot, None, root_w['croot'])

        # ---------- message pass ----------
        qrr = [0]

        def msg_pass(sched, n_win, src_tab, idx_h, dst_h, win_deps, is_cv, gbufs, root_deps):
            with tc.tile_pool(name="gsb", bufs=gbufs) as gp, \
                 tc.tile_pool(name="aux", bufs=2) as ap_, \
                 tc.tile_pool(name="oh", bufs=6) as ohp, \
                 tc.tile_pool(name="sums", bufs=4, space="PSUM") as sp, \
                 tc.tile_pool(name="news", bufs=3, space="PSUM") as npp, \
                 tc.tile_pool(name="eout", bufs=4) as ep, \
                 tc.tile_pool(name="stage", bufs=3) as stp:
                for blocks, runs in sched["groups"]:
                    g_t0 = min(ts for (_, ts, _) in runs)
                    g_t1 = max(ts + n for (_, ts, n) in runs)
                    idx_sb = ap_.tile([128, (g_t1 - g_t0) * 8], I16, tag="idx")
                    nc.sync.dma_start(out=idx_sb[:], in_=idx_h[:, g_t0 * 8:g_t1 * 8])
                    dst_sb = ap_.tile([128, g_t1 - g_t0], F32, tag="dst")
                    nc.sync.dma_start(out=dst_sb[:], in_=dst_h[:, g_t0:g_t1])
                    b0 = blocks[0]
                    nb = len(blocks)
                    rec_h = recv if is_cv else recc
                    rec_sb = ap_.tile([64, 128 * 30], F32, tag="rec")
                    nc.sync.dma_start(
                        out=rec_sb[:, :nb * 128],
                        in_=rec_h[0:1, b0 * 128:(b0 + nb) * 128].partition_broadcast(64).squeeze(1))
                    bs_sb = None
                    if not is_cv:
                        bs_sb = ap_.tile([64, 128 * 30], F32, tag="bs")
                        nc.sync.dma_start(
                            out=bs_sb[:, :nb * 128],
                            in_=bsum[0:1, b0 * 128:(b0 + nb) * 128].partition_broadcast(64).squeeze(1))
                    chunk_tiles = {}
                    for (w, ts, n) in runs:
                        s = ts
                        while s < ts + n:
                            m = min(GCHUNK, ts + n - s)
                            g = gp.tile([128, GCHUNK, 128], BF16, tag="g")
                            gi = nc.gpsimd.dma_gather(
                                out_ap=g[:, :m, :],
                                in_ap=src_tab[w * WIN:(w + 1) * WIN, :],
                                idxs_ap=idx_sb[:, (s - g_t0) * 8:(s - g_t0 + m) * 8],
                                num_idxs=m * 128, num_idxs_reg=m * 128,
                                elem_size=128, single_packet=False,
                                queue_num=qrr[0] % 4)
                            qrr[0] += 1
                            for dep in win_deps[w]:
                                tile.add_dep_helper(gi.ins, dep, reason="tab->gather")
                            for t in range(s, s + m):
                                chunk_tiles[t] = (g, s)
                            s += m
                    for b in blocks:
                        ps = sp.tile([128 if is_cv else 64, 128], F32, tag="ps")
                        ntl = int(sum(sched["T"][b, w] for w in range(n_win)))
                        if ntl == 0:
                            continue
                        done = 0
                        for w in range(n_win):
                            t0 = int(sched["tile_of"][b, w])
                            for t in range(t0, t0 + int(sched["T"][b, w])):
                                g, base = chunk_tiles[t]
                                oh = ohp.tile([128, 128], BF16, tag="oh")
                                nc.vector.tensor_scalar(
                                    out=oh[:], in0=t_iota[:],
                                    scalar1=dst_sb[:, t - g_t0:t - g_t0 + 1],
                                    scalar2=None, op0=ISEQ)
                                lhs = g[:, t - base, :] if is_cv else g[:, t - base, 0:64]
                                done += 1
                                nc.tensor.matmul(ps[:], lhsT=lhs, rhs=oh[:],
                                                 start=(done == 1), stop=(done == ntl))
                        c0, c1_ = b * 128, (b + 1) * 128
                        if is_cv:
                            meanA = ep.tile([64, 128], BF16, tag="meanA")
                            nc.vector.tensor_tensor(
                                out=meanA[:], in0=ps[0:64, :],
                                in1=rec_sb[:, (b - b0) * 128:(b - b0 + 1) * 128], op=MULT)
                            meanB = ep.tile([64, 128], BF16, tag="meanB")
                            nc.vector.tensor_tensor(
                                out=meanB[:], in0=ps[64:128, :],
                                in1=rec_sb[:, (b - b0) * 128:(b - b0 + 1) * 128], op=MULT)
                            xr = ep.tile([64, 128], BF16, tag="xr")
                            xri = nc.sync.dma_start(out=xr[:], in_=vroot[:, c0:c1_])
                            for dep in root_deps:
                                tile.add_dep_helper(xri.ins, dep, reason="root->xr")
                            np1 = npp.tile([64, 128], F32, tag="np")
                            nc.tensor.matmul(np1[:], lhsT=t_w["wl_cv0"][:], rhs=meanA[:],
                                             start=True, stop=False)
                            nc.tensor.matmul(np1[:], lhsT=t_w["wr_cv0"][:], rhs=xr[:],
                                             start=False, stop=True)
                            v1 = ep.tile([64, 128], BF16, tag="v1")
                            nc.scalar.activation(v1[:], np1[:], RELU, bias=t_bl["bl_cv0"][:])
                            np2 = npp.tile([64, 128], F32, tag="np")
                            nc.tensor.matmul(np2[:], lhsT=t_w["wl_cv1"][:], rhs=meanB[:],
                                             start=True, stop=False)
                            nc.tensor.matmul(np2[:], lhsT=t_w["wr_cv1"][:], rhs=v1[:],
                                             start=False, stop=True)
                            vo = stp.tile([64, 128], F32, tag="vo")
                            nc.scalar.activation(vo[:], np2[:], RELU, bias=t_bl["bl_cv1"][:])
                            nc.sync.dma_start(
                                out=out[(b % 2) * 64:(b % 2) * 64 + 64,
                                        (b // 2) * 128:(b // 2) * 128 + 128],
                                in_=vo[:])
                        else:
                            tmp = ep.tile([64, 128], F32, tag="tmp")
                            nc.vector.tensor_scalar(
                                out=tmp[:], in0=bs_sb[:, (b - b0) * 128:(b - b0 + 1) * 128],
                                scalar1=t_brkf[:], scalar2=None, op0=MULT)
                            s2 = ep.tile([64, 128], F32, tag="s2")
                            nc.vector.tensor_tensor(out=s2[:], in0=ps[:], in1=tmp[:], op=ADD)
                            mean = ep.tile([64, 128], BF16, tag="mean")
                            nc.vector.tensor_tensor(
                                out=mean[:], in0=s2[:],
                                in1=rec_sb[:, (b - b0) * 128:(b - b0 + 1) * 128], op=MULT)
                            xr = ep.tile([64, 128], BF16, tag="xr")
                            xri = nc.sync.dma_start(out=xr[:], in_=croot[:, c0:c1_])
                            for dep in root_deps:
                                tile.add_dep_helper(xri.ins, dep, reason="root->xr")
                            np1 = npp.tile([64, 128], F32, tag="np")
                            nc.tensor.matmul(np1[:], lhsT=t_w["wl_vc"][:], rhs=mean[:],
                                             start=True, stop=False)
                            nc.tensor.matmul(np1[:], lhsT=t_w["wr_vc"][:], rhs=xr[:],
                                             start=False, stop=True)
                            c1t = ep.tile([64, 128], BF16, tag="c1t")
                            nc.scalar.activation(c1t[:], np1[:], RELU, bias=t_bl["bl_vc"][:])
                            tp = npp.tile([128, 64], BF16, tag="np")
                            nc.tensor.transpose(out=tp[:], in_=c1t[:], identity=t_ident[:])
                            nm = stp.tile([128, 64], BF16, tag="nm")
                            nc.scalar.activation(nm[:], tp[:], COPY)
                            ag_writes.append(
                                nc.sync.dma_start(out=ag_in[c0:c1_, :], in_=nm[:]).ins)

        with nc.named_scope("vc"):
            msg_pass(svc, WV, var_tab, vc_idx, vc_dst, vwin_w, is_cv=False, gbufs=12, root_deps=root_w['croot'])

        # debug copy of ag_in
        with tc.tile_pool(name="dbgp", bufs=2) as dp:
            for r0 in range(0, SC, 4096):
                m = min(4096, SC - r0)
                t = dp.tile([128, 32, 64], BF16, tag="d")
                rd = nc.sync.dma_start(
                    out=t[:, :m // 128, :],
                    in_=ag_in[r0:r0 + m, :].rearrange("(a p) f -> p a f", p=128))
                for wi in ag_writes:
                    tile.add_dep_helper(rd.ins, wi, reason="agin->dbg")
                nc.sync.dma_start(
                    out=dbg[r0:r0 + m, :].rearrange("(a p) f -> p a f", p=128),
                    in_=t[:, :m // 128, :])

        if "C" in phases:
          with nc.named_scope("ag"):
            coll = nc.gpsimd.collective_compute(
                "AllGather", mybir.AluOpType.bypass,
                ins=[ag_in[:]], outs=[ag_out[:]],
                replica_groups=[list(range(NCORE))])
            for wi in ag_writes:
                tile.add_dep_helper(coll.ins, wi, reason="agin->coll")
            with tc.tile_pool(name="spread", bufs=3) as spp:
                for w in range(WC):
                    for r0 in range(w * WIN, (w + 1) * WIN, 4096):
                        m = min(4096, (w + 1) * WIN - r0)
                        st = spp.tile([128, 32, 64], BF16, tag="st")
                        rd = nc.sync.dma_start(
                            out=st[:, :m // 128, :],
                            in_=ag_out[r0:r0 + m, :].rearrange("(a p) f -> p a f", p=128))
                        tile.add_dep_helper(rd.ins, coll.ins, reason="coll->spread")
                        di = nc.sync.dma_start(
                            out=cv_tab[r0:r0 + m, 64:128].rearrange("(a p) f -> p a f", p=128),
                            in_=st[:, :m // 128, :])
                        cwin_w[w].append(di.ins)
          with nc.named_scope("cv"):
                msg_pass(scv, WC, cv_tab, cv_idx, cv_dst, cwin_w, is_cv=True, gbufs=9, root_deps=root_w['vroot'])

    nc.finalize()
    return nc


def wrap_idx(flat):
    w = flat.reshape(-1, 16).T
    return np.ascontiguousarray(np.tile(w, (8, 1)))


def in_map(P, core):
    s = P
    return {
        "cxT": bf(np.vstack([s["cons_xT"], np.ones((1, CP), np.float32)])),
        "vxT": bf(np.vstack([s["var_xT"], np.ones((1, VP), np.float32)])),
        "o_vxT": bf(np.vstack([s["var_xT"][:, core * SV:(core + 1) * SV],
                               np.ones((1, SV), np.float32)])),
        "o_cxT": bf(np.vstack([s["cons_xT"][:, core * SC:(core + 1) * SC],
                               np.ones((1, SC), np.float32)])),
        "o_brk": bf(s["breakT"][:, core * SV:(core + 1) * SV]),
        "c_l1": bf(np.vstack([s["cW1f"], s["cb1f"][None, :]])),
        "c_l2": bf(np.vstack([s["cW2"], s["cb2"][None, :]])),
        "v_l1": bf(np.vstack([s["vW1f"], s["vb1f"][None, :]])),
        "v_l2": bf(np.vstack([s["vW2"], s["vb2"][None, :]])),
        "wl_vc": bf(s["Wl_vc"][0]), "wr_vc": bf(s["Wr_vc"][0]),
        "wl_cv0": bf(s["Wl_cv"][0]), "wr_cv0": bf(s["Wr_cv"][0]),
        "wl_cv1": bf(s["Wl_cv"][1]), "wr_cv1": bf(s["Wr_cv"][1]),
        "bl_vc": np.ascontiguousarray(s["bl_vc"][0][:, None], dtype=np.float32),
        "bl_cv0": np.ascontiguousarray(s["bl_cv"][0][:, None], dtype=np.float32),
        "bl_cv1": np.ascontiguousarray(s["bl_cv"][1][:, None], dtype=np.float32),
        "brkw_b": bf(s["breakW"][0][:, None]),
        "brkw_f": np.ascontiguousarray(s["breakW"][0][:, None], dtype=np.float32),
        "iota": bf(np.arange(128, dtype=np.float32)[None, :]),
        "recv": np.ascontiguousarray(s["recip_v"][core][None, :]),
        "recc": np.ascontiguousarray(s["recip_c"][core][None, :]),
        "bsum": np.ascontiguousarray(s["bsum_c"][core][None, :]),
        "vc_idx": wrap_idx(s["sched_vc"]["idx16"][core]),
        "vc_dst": np.ascontiguousarray(s["sched_vc"]["dstloc"][core].reshape(-1, 128).T),
        "cv_idx": wrap_idx(s["sched_cv"]["idx16"][core]),
        "cv_dst": np.ascontiguousarray(s["sched_cv"]["dstloc"][core].reshape(-1, 128).T),
    }


def unpack_out(outs_per_core, pid_v):
    """outs: list of [128, OUT_W] f32 -> full [NVAR, 64] unpermuted."""
    var2T = np.zeros((64, NCORE * SV), dtype=np.float32)
    for k, o in enumerate(outs_per_core):
        o = o.reshape(128, OUT_W // 128, 128)
        base = k * SV
        for half in range(2):
            blocks = o[half * 64:(half + 1) * 64]    # [64, npair, 128]
            npair = blocks.shape[1]
            idxs = (np.arange(npair) * 2 + half) * 128
            for i, c in enumerate(idxs):
                var2T[:, base + c:base + c + 128] = blocks[:, i, :]
    return var2T.T[pid_v]


# ---------------- top-level kernel entry ----------------
_CACHE = {}


def kernel(**inputs):
    import numpy as _np
    key = "k"
    if key not in _CACHE:
        P = preprocess(inputs)
        nc = build(P, phases="EVC")
        _CACHE[key] = (P, nc)
    P, nc = _CACHE[key]
    from concourse.bass_utils import run_bass_kernel_spmd
    in_maps = [in_map(P, k) for k in range(NCORE)]
    res = run_bass_kernel_spmd(nc, in_maps, core_ids=list(range(NCORE)))
    outs = [res.results[k]["out"] for k in range(NCORE)]
    return unpack_out(outs, P["pid_v"]).astype(_np.float32)

